# revision 12
# baseline (speedup 1.0000x reference)
"""Multi-head cross-attention kernel for Trainium2, 8 NeuronCores.

Problem: nn_MultiHeadAttention (H=32 heads, B=8, Lq=Lk=1024, E=128, D=512).

    keys   = einsum('bkd,hde->hbke', states, Wk) + bk
    values = einsum('bkd,hde->hbke', states, Wv) + bv
    attn   = softmax(einsum('bqe,hbke->hbqk', query, keys) / sqrt(E))
    ctx    = einsum('hbqk,hbke->hbqe', attn, values)  -> concat heads
    out    = ctx @ Wo + bo

Sharding: data parallel over batch B=8 -> one batch element per core; no
collectives needed.  Per-core dataflow (all matmuls fp32r, full PE rate):

  K^T[h] = Wk[h]^T-chunks @ states^T          [E=128p, Lk]     (psum, +bk on copy)
  V[4h]  = states^T-blocks @ Wv-packed        [Lk-chunk, 4*E]  (4 heads at once)
  S^T    = K^T-block @ query^T                [Lk-chunk p, Lq] (chunked over Lk)
  P      = exp(S^T * 1/sqrt(E))               (ACT, no max-subtraction: scores
                                               are O(4) so exp is safe in fp32)
  rowsum = ones[128,128] @ P-chunks           [128, Lq] (accum, all rows equal)
  ctx^T  = V-chunk @ P-chunks (accum)         [E, Lq]
  ctxn   = ctx^T * reciprocal(rowsum)         (DVE)
  out^T += Wo[h] @ ctxn                       [E, Lq]  (SBUF accumulation)

bv is folded into the output bias on the host (softmax rows sum to 1, so
ctx = attn@(states@Wv) + bv exactly): bo' = bo + sum_h bv[h] @ Wo[h].
"""

import numpy as np

import concourse.bass as bass
import concourse.mybir as mybir
import concourse.tile as tile
from concourse import bacc
from concourse.bass_utils import run_bass_kernel_spmd

H, E, D = 32, 128, 512
B, LQ, LK = 8, 1024, 1024
NDC = D // 128    # 4 contraction chunks for the projections
NLK = LK // 128   # 8 key chunks
HPG = 4           # heads per group for the packed V computation
NG = H // HPG
SCALE = 1.0 / float(np.sqrt(E))

F32 = mybir.dt.float32
F32R = mybir.dt.float32r
EXP = mybir.ActivationFunctionType.Exp

N_CORES = 8


def _build_kernel(tc, qT, sT, wk, wv, wo, bkT, bo2, ones, outT):
    nc = tc.nc
    with (
        tc.tile_pool(name="const", bufs=1) as cpool,
        tc.tile_pool(name="wkp", bufs=2) as wkp,
        tc.tile_pool(name="wvp", bufs=2) as wvp,
        tc.tile_pool(name="wop", bufs=2) as wop,
        tc.tile_pool(name="ktp", bufs=2) as ktp,
        tc.tile_pool(name="vp", bufs=2) as vpool,
        tc.tile_pool(name="pp", bufs=4) as ppool,
        tc.tile_pool(name="normp", bufs=2) as npool,
        tc.tile_pool(name="ps_sh", bufs=2, space="PSUM") as ps_sh,
        tc.tile_pool(name="ps_acc", bufs=1, space="PSUM") as ps_acc,
    ):
        # ---- resident inputs ----
        q_sb = cpool.tile([E, LQ], F32R)
        nc.sync.dma_start(q_sb[:], qT[:])
        st_sb = cpool.tile([128, NDC, LK], F32R)
        for c in range(NDC):
            nc.sync.dma_start(st_sb[:, c, :], sT[c * 128:(c + 1) * 128, :])
        ones_sb = cpool.tile([128, 128], F32R)
        nc.sync.dma_start(ones_sb[:], ones[:])
        bk_sb = cpool.tile([E, H], F32)
        nc.sync.dma_start(bk_sb[:], bkT[:])
        bo2_sb = cpool.tile([E, 1], F32)
        nc.sync.dma_start(bo2_sb[:], bo2[:])
        out_acc = cpool.tile([E, LQ], F32)

        # proj state carried across heads so head h+1's K/S matmuls can be
        # emitted before head h's projection (keeps PE busy during the DVE
        # normalization of head h).
        pending = {}  # h -> ctxn tile

        def emit_pending_proj():
            if not pending:
                return
            (h, ctxn_sb), = pending.items()
            pending.clear()
            wo_sb = wop.tile([E, E], F32R, tag="wo", name="wo_sb")
            nc.sync.dma_start(wo_sb[:], wo[h * E:(h + 1) * E, :])
            ps_p = ps_sh.tile([E, LQ], F32, tag="sh", name="ps_p")
            for half in range(2):
                sl = bass.ts(half, 512)
                nc.tensor.matmul(ps_p[:, sl], (wo_sb[:]), (ctxn_sb[:, sl]),
                                 start=True, stop=True)
            if h == 0:
                nc.vector.tensor_scalar_add(out_acc[:], ps_p[:], bo2_sb[:, 0:1])
            else:
                nc.vector.tensor_add(out_acc[:], out_acc[:], ps_p[:])

        for g in range(NG):
            # ---- packed V for the 4 heads of this group ----
            wv_sb = wvp.tile([128, NDC, HPG * E], F32R, tag="wv", name="wv_sb")
            for c in range(NDC):
                nc.sync.dma_start(
                    wv_sb[:, c, :],
                    wv[c * 128:(c + 1) * 128, g * HPG * E:(g + 1) * HPG * E])
            v_sb = vpool.tile([128, NLK, HPG * E], F32R, tag="v", name="v_sb")
            for lk in range(NLK):
                ps_v = ps_sh.tile([128, HPG * E], F32, tag="sh", name="ps_v")
                for c in range(NDC):
                    nc.tensor.matmul(
                        ps_v[:], (st_sb[:, c, lk * 128:(lk + 1) * 128]),
                        (wv_sb[:, c, :]), start=(c == 0), stop=(c == NDC - 1))
                nc.vector.tensor_copy(v_sb[:, lk, :], ps_v[:])

            for hh in range(HPG):
                h = g * HPG + hh
                # ---- K^T = per-head key projection ----
                wk_sb = wkp.tile([128, NDC, E], F32R, tag="wk", name="wk_sb")
                for c in range(NDC):
                    nc.sync.dma_start(wk_sb[:, c, :], wk[h, c * 128:(c + 1) * 128, :])
                # two separate psum tiles per half so the DVE bias-add of
                # half 0 never WAR-serializes against the PE writing half 1
                kt_sb = ktp.tile([E, LK], F32R, tag="kt", name="kt_sb")
                for half in range(2):
                    sl = bass.ts(half, 512)
                    ps_k = ps_sh.tile([E, 512], F32, tag="sh", name="ps_k")
                    for c in range(NDC):
                        nc.tensor.matmul(ps_k[:], (wk_sb[:, c, :]),
                                         (st_sb[:, c, sl]),
                                         start=(c == 0), stop=(c == NDC - 1))
                    nc.vector.tensor_scalar_add(kt_sb[:, sl], ps_k[:],
                                                bk_sb[:, h:h + 1])

                # ---- attention, software-pipelined one S-chunk ahead ----
                ps_r = ps_acc.tile([128, LQ], F32, tag="r", name="ps_r")
                ps_c = ps_acc.tile([E, LQ], F32, tag="c", name="ps_c")

                def emit_s(lk):
                    ps_s = ps_sh.tile([128, LQ], F32, tag="sh", name="ps_s")
                    for half in range(2):
                        sl = bass.ts(half, 512)
                        nc.tensor.matmul(ps_s[:, sl],
                                         (kt_sb[:, lk * 128:(lk + 1) * 128]),
                                         (q_sb[:, sl]), start=True, stop=True)
                    p_sb = ppool.tile([128, LQ], F32R, tag="p", name="p_sb")
                    nc.scalar.activation(p_sb[:], ps_s[:], EXP, scale=SCALE)
                    return p_sb

                p_next = emit_s(0)
                first = True
                for lk in range(NLK):
                    p_cur = p_next
                    if lk + 1 < NLK:
                        p_next = emit_s(lk + 1)
                    if first:
                        # projection of the previous head slots in here, after
                        # this head's first two S chunks keep the PE busy
                        emit_pending_proj()
                        first = False
                    for half in range(2):
                        sl = bass.ts(half, 512)
                        nc.tensor.matmul(ps_r[:, sl], (ones_sb[:]),
                                         (p_cur[:, sl]),
                                         start=(lk == 0), stop=(lk == NLK - 1))
                        nc.tensor.matmul(ps_c[:, sl],
                                         (v_sb[:, lk, hh * E:(hh + 1) * E]),
                                         (p_cur[:, sl]),
                                         start=(lk == 0), stop=(lk == NLK - 1))

                # ---- normalization (DVE; overlaps next head's K matmuls) ----
                recip_sb = npool.tile([128, LQ], F32, tag="recip", name="recip_sb")
                nc.vector.reciprocal_approx_fast(recip_sb[:], ps_r[:])
                ctxn_sb = npool.tile([E, LQ], F32R, tag="ctxn", name="ctxn_sb")
                nc.vector.tensor_mul(ctxn_sb[:], ps_c[:], recip_sb[:])
                pending[h] = ctxn_sb

        emit_pending_proj()
        nc.sync.dma_start(outT[:], out_acc[:])


def build_program():
    nc = bacc.Bacc("TRN2", target_bir_lowering=False, debug=False,
                   num_devices=N_CORES)
    qT = nc.dram_tensor("qT", [E, LQ], F32R, kind="ExternalInput").ap()
    sT = nc.dram_tensor("sT", [D, LK], F32R, kind="ExternalInput").ap()
    wk = nc.dram_tensor("wk", [H, D, E], F32R, kind="ExternalInput").ap()
    wv = nc.dram_tensor("wv", [D, H * E], F32R, kind="ExternalInput").ap()
    wo = nc.dram_tensor("wo", [H * E, E], F32R, kind="ExternalInput").ap()
    bkT = nc.dram_tensor("bkT", [E, H], F32, kind="ExternalInput").ap()
    bo2 = nc.dram_tensor("bo2", [E, 1], F32, kind="ExternalInput").ap()
    ones = nc.dram_tensor("ones", [128, 128], F32R, kind="ExternalInput").ap()
    outT = nc.dram_tensor("outT", [E, LQ], F32, kind="ExternalOutput").ap()

    with tile.TileContext(nc) as tc:
        _build_kernel(tc, qT, sT, wk, wv, wo, bkT, bo2, ones, outT)
    nc.compile()
    return nc


def _round_f32r(a):
    """Round fp32 -> fp32r (11-bit mantissa, low 12 bits zero), RN-even.

    The PE's fp32r datapath keeps sign+8exp+11mantissa; the BIR verifier
    requires fp32r matmul operands to be pre-rounded, and rounding on the
    host gives round-to-nearest instead of hardware truncation.
    """
    b = np.ascontiguousarray(a, dtype=np.float32).view(np.uint32)
    b = b + 0x7FF + ((b >> 12) & 1)
    b &= np.uint32(0xFFFFF000)
    return b.view(np.float32)


def make_in_maps(query, states, Wk, bk, Wv, bv, Wo, bo):
    """Shard the full inputs into per-core input maps (host-side prep)."""
    wv_packed = np.ascontiguousarray(
        np.transpose(Wv, (1, 0, 2)).reshape(D, H * E))
    # fold bv through the output projection: softmax rows sum to 1
    bo2 = bo.astype(np.float64).copy()
    for h in range(H):
        bo2 += bv[h].astype(np.float64) @ Wo[h * E:(h + 1) * E].astype(np.float64)
    bo2 = bo2.astype(np.float32).reshape(E, 1)
    bkT = np.ascontiguousarray(bk.T)
    wk_c = _round_f32r(Wk)
    wo_c = _round_f32r(Wo)
    wv_packed = _round_f32r(wv_packed)

    in_maps = []
    for b in range(B):
        in_maps.append({
            "qT": _round_f32r(query[b].T),
            "sT": _round_f32r(states[b].T),
            "wk": wk_c,
            "wv": wv_packed,
            "wo": wo_c,
            "bkT": bkT,
            "bo2": bo2,
            "ones": np.ones((128, 128), dtype=np.float32),
        })
    return in_maps


_PROGRAM_CACHE = {}


def _get_program():
    if "nc" not in _PROGRAM_CACHE:
        _PROGRAM_CACHE["nc"] = build_program()
    return _PROGRAM_CACHE["nc"]


def kernel(query, states, Wk, bk, Wv, bv, Wo, bo, _trace=False, _tmpdir=None):
    nc = _get_program()
    in_maps = make_in_maps(query, states, Wk, bk, Wv, bv, Wo, bo)
    res = run_bass_kernel_spmd(nc, in_maps, core_ids=list(range(N_CORES)),
                               trace=_trace, tmpdir=_tmpdir)
    out = np.stack([res.results[b]["outT"].T for b in range(B)])
    out = np.ascontiguousarray(out.astype(np.float32))
    if _trace:
        kernel.last_exec_time_ns = res.exec_time_ns
        kernel.last_results = res
    return out


if __name__ == "__main__":
    rng = np.random.default_rng(0)
    inputs = {
        "query": rng.standard_normal((B, LQ, E), dtype=np.float32),
        "states": rng.standard_normal((B, LK, D), dtype=np.float32),
        "Wk": rng.uniform(-0.04, 0.04, (H, D, E)).astype(np.float32),
        "bk": rng.uniform(-0.04, 0.04, (H, E)).astype(np.float32),
        "Wv": rng.uniform(-0.04, 0.04, (H, D, E)).astype(np.float32),
        "bv": rng.uniform(-0.04, 0.04, (H, E)).astype(np.float32),
        "Wo": rng.uniform(-0.015, 0.015, (H * E, E)).astype(np.float32),
        "bo": rng.uniform(-0.015, 0.015, (E,)).astype(np.float32),
    }
    out = kernel(**inputs)
    print(out.shape, out.dtype)


# revision 13
# speedup vs baseline: 1.0856x; 1.0856x over previous
"""Multi-head cross-attention kernel for Trainium2, 8 NeuronCores.

Problem: nn_MultiHeadAttention (H=32 heads, B=8, Lq=Lk=1024, E=128, D=512).

    keys   = einsum('bkd,hde->hbke', states, Wk) + bk
    values = einsum('bkd,hde->hbke', states, Wv) + bv
    attn   = softmax(einsum('bqe,hbke->hbqk', query, keys) / sqrt(E))
    ctx    = einsum('hbqk,hbke->hbqe', attn, values)  -> concat heads
    out    = ctx @ Wo + bo

Sharding: data parallel over batch B=8 -> one batch element per core; no
collectives needed.  Per-core dataflow (all matmuls fp32r, full PE rate):

  K^T[h] = Wk[h]^T-chunks @ states^T          [E=128p, Lk]     (psum, +bk on copy)
  V[4h]  = states^T-blocks @ Wv-packed        [Lk-chunk, 4*E]  (4 heads at once)
  S^T    = K^T-block @ query^T                [Lk-chunk p, Lq] (chunked over Lk)
  P      = exp(S^T * 1/sqrt(E))               (ACT, no max-subtraction: scores
                                               are O(4) so exp is safe in fp32)
  rowsum = ones[128,128] @ P-chunks           [128, Lq] (accum, all rows equal)
  ctx^T  = V-chunk @ P-chunks (accum)         [E, Lq]
  ctxn   = ctx^T * reciprocal(rowsum)         (DVE)
  out^T += Wo[h] @ ctxn                       [E, Lq]  (SBUF accumulation)

bv is folded into the output bias on the host (softmax rows sum to 1, so
ctx = attn@(states@Wv) + bv exactly): bo' = bo + sum_h bv[h] @ Wo[h].
"""

import numpy as np

import concourse.bass as bass
import concourse.mybir as mybir
import concourse.tile as tile
from concourse import bacc
from concourse.bass_utils import run_bass_kernel_spmd

H, E, D = 32, 128, 512
B, LQ, LK = 8, 1024, 1024
NDC = D // 128    # 4 contraction chunks for the projections
NLK = LK // 128   # 8 key chunks
HPG = 4           # heads per group for the packed V computation
NG = H // HPG
SCALE = 1.0 / float(np.sqrt(E))

F32 = mybir.dt.float32
F32R = mybir.dt.float32r
EXP = mybir.ActivationFunctionType.Exp
COPY = mybir.ActivationFunctionType.Copy

N_CORES = 8


def _build_kernel(tc, qT, sT, wk, wv, wo, bo2, ones, outT):
    nc = tc.nc
    with (
        tc.tile_pool(name="const", bufs=1) as cpool,
        tc.tile_pool(name="wkp", bufs=2) as wkp,
        tc.tile_pool(name="wvp", bufs=2) as wvp,
        tc.tile_pool(name="wop", bufs=2) as wop,
        tc.tile_pool(name="ktp", bufs=2) as ktp,
        tc.tile_pool(name="vp", bufs=2) as vpool,
        tc.tile_pool(name="pp", bufs=4) as ppool,
        tc.tile_pool(name="normp", bufs=2) as npool,
        tc.tile_pool(name="ps_sh", bufs=2, space="PSUM") as ps_sh,
        tc.tile_pool(name="ps_acc", bufs=1, space="PSUM") as ps_acc,
    ):
        # ---- resident inputs ----
        q_sb = cpool.tile([E, LQ], F32R)
        nc.sync.dma_start(q_sb[:], qT[:])
        st_sb = cpool.tile([128, NDC, LK], F32R)
        for c in range(NDC):
            nc.sync.dma_start(st_sb[:, c, :], sT[c * 128:(c + 1) * 128, :])
        ones_sb = cpool.tile([128, 128], F32R)
        nc.sync.dma_start(ones_sb[:], ones[:])
        bo2_sb = cpool.tile([E, 1], F32)
        nc.sync.dma_start(bo2_sb[:], bo2[:])
        out_acc = cpool.tile([E, LQ], F32)

        # proj state carried across heads so head h+1's K/S matmuls can be
        # emitted before head h's projection (keeps PE busy during the DVE
        # normalization of head h).
        pending = {}  # h -> ctxn tile

        def emit_pending_proj():
            if not pending:
                return
            (h, ctxn_sb), = pending.items()
            pending.clear()
            wo_sb = wop.tile([E, E], F32R, tag="wo", name="wo_sb")
            nc.sync.dma_start(wo_sb[:], wo[h * E:(h + 1) * E, :])
            ps_p = ps_sh.tile([E, LQ], F32, tag="sh", name="ps_p")
            for half in range(2):
                sl = bass.ts(half, 512)
                nc.tensor.matmul(ps_p[:, sl], (wo_sb[:]), (ctxn_sb[:, sl]),
                                 start=True, stop=True)
            if h == 0:
                nc.vector.tensor_scalar_add(out_acc[:], ps_p[:], bo2_sb[:, 0:1])
            else:
                nc.vector.tensor_add(out_acc[:], out_acc[:], ps_p[:])

        for g in range(NG):
            # ---- packed V for the 4 heads of this group ----
            wv_sb = wvp.tile([128, NDC, HPG * E], F32R, tag="wv", name="wv_sb")
            for c in range(NDC):
                nc.sync.dma_start(
                    wv_sb[:, c, :],
                    wv[c * 128:(c + 1) * 128, g * HPG * E:(g + 1) * HPG * E])
            v_sb = vpool.tile([128, NLK, HPG * E], F32R, tag="v", name="v_sb")
            for lk in range(NLK):
                ps_v = ps_sh.tile([128, HPG * E], F32, tag="sh", name="ps_v")
                for c in range(NDC):
                    nc.tensor.matmul(
                        ps_v[:], (st_sb[:, c, lk * 128:(lk + 1) * 128]),
                        (wv_sb[:, c, :]), start=(c == 0), stop=(c == NDC - 1))
                nc.vector.tensor_copy(v_sb[:, lk, :], ps_v[:])

            for hh in range(HPG):
                h = g * HPG + hh
                # ---- K^T = per-head key projection ----
                wk_sb = wkp.tile([128, NDC, E], F32R, tag="wk", name="wk_sb")
                for c in range(NDC):
                    nc.sync.dma_start(wk_sb[:, c, :], wk[h, c * 128:(c + 1) * 128, :])
                # two separate psum tiles per half so the ACT copy of
                # half 0 never WAR-serializes against the PE writing half 1.
                # bk is dropped: softmax(S + const-per-row) == softmax(S).
                kt_sb = ktp.tile([E, LK], F32R, tag="kt", name="kt_sb")
                for half in range(2):
                    sl = bass.ts(half, 512)
                    ps_k = ps_sh.tile([E, 512], F32, tag="sh", name="ps_k")
                    for c in range(NDC):
                        nc.tensor.matmul(ps_k[:], (wk_sb[:, c, :]),
                                         (st_sb[:, c, sl]),
                                         start=(c == 0), stop=(c == NDC - 1))
                    nc.scalar.activation(kt_sb[:, sl], ps_k[:], COPY)

                # ---- attention, software-pipelined one S-chunk ahead ----
                ps_r = ps_acc.tile([128, LQ], F32, tag="r", name="ps_r")
                ps_c = ps_acc.tile([E, LQ], F32, tag="c", name="ps_c")

                def emit_s(lk):
                    ps_s = ps_sh.tile([128, LQ], F32, tag="sh", name="ps_s")
                    for half in range(2):
                        sl = bass.ts(half, 512)
                        nc.tensor.matmul(ps_s[:, sl],
                                         (kt_sb[:, lk * 128:(lk + 1) * 128]),
                                         (q_sb[:, sl]), start=True, stop=True)
                    p_sb = ppool.tile([128, LQ], F32R, tag="p", name="p_sb")
                    nc.scalar.activation(p_sb[:], ps_s[:], EXP, scale=SCALE)
                    return p_sb

                p_next = emit_s(0)
                first = True
                for lk in range(NLK):
                    p_cur = p_next
                    if lk + 1 < NLK:
                        p_next = emit_s(lk + 1)
                    if first:
                        # projection of the previous head slots in here, after
                        # this head's first two S chunks keep the PE busy
                        emit_pending_proj()
                        first = False
                    for half in range(2):
                        sl = bass.ts(half, 512)
                        nc.tensor.matmul(ps_r[:, sl], (ones_sb[:]),
                                         (p_cur[:, sl]),
                                         start=(lk == 0), stop=(lk == NLK - 1))
                        nc.tensor.matmul(ps_c[:, sl],
                                         (v_sb[:, lk, hh * E:(hh + 1) * E]),
                                         (p_cur[:, sl]),
                                         start=(lk == 0), stop=(lk == NLK - 1))

                # ---- normalization (DVE; overlaps next head's K matmuls) ----
                recip_sb = npool.tile([128, LQ], F32, tag="recip", name="recip_sb")
                nc.vector.reciprocal_approx_fast(recip_sb[:], ps_r[:])
                ctxn_sb = npool.tile([E, LQ], F32R, tag="ctxn", name="ctxn_sb")
                nc.vector.tensor_mul(ctxn_sb[:], ps_c[:], recip_sb[:])
                pending[h] = ctxn_sb

        emit_pending_proj()
        nc.sync.dma_start(outT[:], out_acc[:])


def build_program():
    nc = bacc.Bacc("TRN2", target_bir_lowering=False, debug=False,
                   num_devices=N_CORES)
    qT = nc.dram_tensor("qT", [E, LQ], F32R, kind="ExternalInput").ap()
    sT = nc.dram_tensor("sT", [D, LK], F32R, kind="ExternalInput").ap()
    wk = nc.dram_tensor("wk", [H, D, E], F32R, kind="ExternalInput").ap()
    wv = nc.dram_tensor("wv", [D, H * E], F32R, kind="ExternalInput").ap()
    wo = nc.dram_tensor("wo", [H * E, E], F32R, kind="ExternalInput").ap()
    bo2 = nc.dram_tensor("bo2", [E, 1], F32, kind="ExternalInput").ap()
    ones = nc.dram_tensor("ones", [128, 128], F32R, kind="ExternalInput").ap()
    outT = nc.dram_tensor("outT", [E, LQ], F32, kind="ExternalOutput").ap()

    with tile.TileContext(nc) as tc:
        _build_kernel(tc, qT, sT, wk, wv, wo, bo2, ones, outT)
    nc.compile()
    return nc


def _round_f32r(a):
    """Round fp32 -> fp32r (11-bit mantissa, low 12 bits zero), RN-even.

    The PE's fp32r datapath keeps sign+8exp+11mantissa; the BIR verifier
    requires fp32r matmul operands to be pre-rounded, and rounding on the
    host gives round-to-nearest instead of hardware truncation.
    """
    b = np.ascontiguousarray(a, dtype=np.float32).view(np.uint32)
    b = b + 0x7FF + ((b >> 12) & 1)
    b &= np.uint32(0xFFFFF000)
    return b.view(np.float32)


def make_in_maps(query, states, Wk, bk, Wv, bv, Wo, bo):
    """Shard the full inputs into per-core input maps (host-side prep)."""
    wv_packed = np.ascontiguousarray(
        np.transpose(Wv, (1, 0, 2)).reshape(D, H * E))
    # fold bv through the output projection: softmax rows sum to 1
    bo2 = bo.astype(np.float64).copy()
    for h in range(H):
        bo2 += bv[h].astype(np.float64) @ Wo[h * E:(h + 1) * E].astype(np.float64)
    bo2 = bo2.astype(np.float32).reshape(E, 1)
    wk_c = _round_f32r(Wk)
    wo_c = _round_f32r(Wo)
    wv_packed = _round_f32r(wv_packed)

    in_maps = []
    for b in range(B):
        in_maps.append({
            "qT": _round_f32r(query[b].T),
            "sT": _round_f32r(states[b].T),
            "wk": wk_c,
            "wv": wv_packed,
            "wo": wo_c,
            "bo2": bo2,
            "ones": np.ones((128, 128), dtype=np.float32),
        })
    return in_maps


_PROGRAM_CACHE = {}


def _get_program():
    if "nc" not in _PROGRAM_CACHE:
        _PROGRAM_CACHE["nc"] = build_program()
    return _PROGRAM_CACHE["nc"]


def kernel(query, states, Wk, bk, Wv, bv, Wo, bo, _trace=False, _tmpdir=None):
    nc = _get_program()
    in_maps = make_in_maps(query, states, Wk, bk, Wv, bv, Wo, bo)
    res = run_bass_kernel_spmd(nc, in_maps, core_ids=list(range(N_CORES)),
                               trace=_trace, tmpdir=_tmpdir)
    out = np.stack([res.results[b]["outT"].T for b in range(B)])
    out = np.ascontiguousarray(out.astype(np.float32))
    if _trace:
        kernel.last_exec_time_ns = res.exec_time_ns
        kernel.last_results = res
    return out


if __name__ == "__main__":
    rng = np.random.default_rng(0)
    inputs = {
        "query": rng.standard_normal((B, LQ, E), dtype=np.float32),
        "states": rng.standard_normal((B, LK, D), dtype=np.float32),
        "Wk": rng.uniform(-0.04, 0.04, (H, D, E)).astype(np.float32),
        "bk": rng.uniform(-0.04, 0.04, (H, E)).astype(np.float32),
        "Wv": rng.uniform(-0.04, 0.04, (H, D, E)).astype(np.float32),
        "bv": rng.uniform(-0.04, 0.04, (H, E)).astype(np.float32),
        "Wo": rng.uniform(-0.015, 0.015, (H * E, E)).astype(np.float32),
        "bo": rng.uniform(-0.015, 0.015, (E,)).astype(np.float32),
    }
    out = kernel(**inputs)
    print(out.shape, out.dtype)
